# revision 6
# baseline (speedup 1.0000x reference)
"""Single-head attention (B=4, S=4096, E=1024, D=64) on 8 TRN2 NeuronCores.

Sharding: pure data-parallel over (batch, query-half): core c handles batch
b = c // 2 and query rows [h*2048, (h+1)*2048) with h = c % 2. Each core
computes Q for its own 2048 rows and K/V for the full 4096 rows of its batch
(inputs are shipped pre-transposed per half, so there is no duplicated DMA).

On-chip dataflow per core (all matmuls on TensorE, fp32 PSUM):
  x^T (bf16, host-pretransposed)  --WqT/WkT/WvT (bf16)-->  Q^T, K^T [64, S],
  V^T -> (PE transpose) -> V natural [k, 64] augmented with a ones column.
  scores^T[k, q] = K^T.T @ Q^T  (fp32r)  -> exp on ScalarE -> P [k, q]
  attn^T[65, q] += V_aug.T @ P  (fp32r; row 64 = softmax denominators)
  final: PE-transpose attn^T tiles, multiply by reciprocal denominators, DMA.
"""

import os

import numpy as np

B, S, E, D = 4, 4096, 1024, 64
HALF = S // 2
N_CORES = 8
SCALE = 1.0 / np.sqrt(D)

NE = E // 128  # 8 e-tiles
NKT = S // 128  # 32 k-tiles
NQT = HALF // 128  # 16 q-tiles per core

_CACHE = {}


def _build():
    if "nc" in _CACHE:
        return _CACHE["nc"]

    from contextlib import ExitStack

    import concourse.bacc as bacc
    import concourse.tile as tile
    from concourse import mybir
    from concourse.masks import make_identity

    FP32 = mybir.dt.float32
    FP32R = mybir.dt.float32r
    BF16 = mybir.dt.bfloat16
    Exp = mybir.ActivationFunctionType.Exp

    nc = bacc.Bacc(
        "TRN2", target_bir_lowering=False, debug=False, num_devices=N_CORES
    )

    xt_q_d = nc.dram_tensor("xt_q", [E, HALF], BF16, kind="ExternalInput").ap()
    xt_o_d = nc.dram_tensor("xt_o", [E, HALF], BF16, kind="ExternalInput").ap()
    wt_d = nc.dram_tensor("wt", [E, 3 * D], BF16, kind="ExternalInput").ap()
    out_d = nc.dram_tensor("out", [HALF, D], FP32, kind="ExternalOutput").ap()

    with tile.TileContext(nc) as tc, ExitStack() as ctx:
        const = ctx.enter_context(tc.tile_pool(name="const", bufs=1))
        big = ctx.enter_context(tc.tile_pool(name="big", bufs=1))
        pp = ctx.enter_context(tc.tile_pool(name="pp", bufs=3))
        sm = ctx.enter_context(tc.tile_pool(name="sm", bufs=4))
        psA = ctx.enter_context(tc.tile_pool(name="psA", bufs=2, space="PSUM"))
        psB = ctx.enter_context(tc.tile_pool(name="psB", bufs=1, space="PSUM"))

        ident = const.tile([128, 128], FP32)
        make_identity(nc, ident)

        xt = big.tile([128, NE, S], BF16)  # x^T; [:, :, :HALF] own q-rows
        wt = big.tile([128, NE, 3 * D], BF16)  # WqT | WkT | WvT
        qt = big.tile([64, HALF], FP32R)  # Q^T
        kt = big.tile([64, S], FP32R)  # K^T
        vt = big.tile([65, S], FP32)  # V^T (staging); row 64 = ones
        vn = big.tile([128, NKT, D + 1], FP32R)  # V natural + ones column
        att_sb = big.tile([65, HALF], FP32)  # attn^T + denominator row
        out_sb = big.tile([128, NQT, D], FP32)

        # --- input DMAs ---
        for et in range(NE):
            nc.sync.dma_start(
                out=xt[:, et, 0:HALF], in_=xt_q_d[et * 128 : (et + 1) * 128, :]
            )
            nc.sync.dma_start(
                out=xt[:, et, HALF:S], in_=xt_o_d[et * 128 : (et + 1) * 128, :]
            )
        nc.sync.dma_start(out=wt[:, :, :], in_=wt_d.rearrange("(t p) d -> p t d", p=128))

        nc.vector.memset(vt[64:65, :], 1.0)

        # --- projections: dst^T[64, s] accumulated over e-tiles ---
        def proj(widx, dst, n_cols):
            for c in range(n_cols // 512):
                acc = psA.tile([128, 1024], FP32, tag="ps")
                for et in range(NE):
                    nc.tensor.matmul(
                        out=acc[0:64, 0:512],
                        lhsT=wt[:, et, widx * D : (widx + 1) * D],
                        rhs=xt[:, et, c * 512 : (c + 1) * 512],
                        start=(et == 0),
                        stop=(et == NE - 1),
                    )
                nc.vector.tensor_copy(
                    out=dst[:, c * 512 : (c + 1) * 512], in_=acc[0:64, 0:512]
                )

        proj(0, qt, HALF)  # Q over own rows only
        proj(1, kt, S)  # K over all rows of the batch
        proj(2, vt[0:64, :], S)  # V over all rows of the batch

        # --- V natural + ones column via PE transpose of V^T_aug tiles ---
        for k in range(NKT):
            tp = psA.tile([128, 1024], FP32, tag="ps")
            nc.tensor.transpose(
                out=tp[0:128, 0:65],
                in_=vt[:, k * 128 : (k + 1) * 128],
                identity=ident[0:65, 0:65],
            )
            nc.vector.tensor_copy(out=vn[:, k, :], in_=tp[0:128, 0:65])

        # --- attention: scores^T -> exp -> P, attn^T accumulation ---
        att_ps = psB.tile([128, HALF], FP32)
        for k in range(NKT):
            for h in range(2):
                sc = psA.tile([128, 1024], FP32, tag="ps")
                for c in range(2):
                    q0 = h * 1024 + c * 512
                    nc.tensor.matmul(
                        out=sc[:, c * 512 : (c + 1) * 512],
                        lhsT=kt[:, k * 128 : (k + 1) * 128],
                        rhs=qt[:, q0 : q0 + 512],
                        start=True,
                        stop=True,
                    )
                p = pp.tile([128, 1024], FP32R)
                nc.scalar.activation(out=p[:, :], in_=sc[:, :], func=Exp, scale=SCALE)
                for c in range(2):
                    q0 = h * 1024 + c * 512
                    nc.tensor.matmul(
                        out=att_ps[0:65, q0 : q0 + 512],
                        lhsT=vn[:, k, :],
                        rhs=p[:, c * 512 : (c + 1) * 512],
                        start=(k == 0),
                        stop=(k == NKT - 1),
                        skip_group_check=True,
                    )

        # --- normalize + transpose back to [q, d] ---
        nc.vector.tensor_copy(out=att_sb[:, :], in_=att_ps[0:65, :])
        for t in range(NQT):
            tp = psA.tile([128, 1024], FP32, tag="ps")
            nc.tensor.transpose(
                out=tp[0:128, 0:65],
                in_=att_sb[:, t * 128 : (t + 1) * 128],
                identity=ident[0:65, 0:65],
            )
            rc = sm.tile([128, 1], FP32)
            nc.vector.reciprocal(out=rc[:, :], in_=tp[0:128, 64:65])
            nc.vector.tensor_scalar_mul(out_sb[:, t, :], tp[0:128, 0:64], rc[:, :])

        nc.sync.dma_start(
            out=out_d.rearrange("(t p) d -> p t d", p=128), in_=out_sb[:, :, :]
        )

    nc.compile()
    _CACHE["nc"] = nc
    return nc


def _make_in_maps(x, Wq, Wk, Wv):
    import ml_dtypes

    bf16 = ml_dtypes.bfloat16
    xT = np.ascontiguousarray(x.transpose(0, 2, 1)).astype(bf16)  # [B, E, S]
    wt = np.concatenate([Wq.T, Wk.T, Wv.T], axis=1).astype(bf16)  # [E, 3D]
    in_maps = []
    for c in range(N_CORES):
        b, h = divmod(c, 2)
        in_maps.append(
            {
                "xt_q": np.ascontiguousarray(xT[b, :, h * HALF : (h + 1) * HALF]),
                "xt_o": np.ascontiguousarray(
                    xT[b, :, (1 - h) * HALF : (2 - h) * HALF]
                ),
                "wt": wt,
            }
        )
    return in_maps


def _run(x, Wq, Wk, Wv, trace=False):
    from concourse.bass_utils import run_bass_kernel_spmd

    nc = _build()
    in_maps = _make_in_maps(x, Wq, Wk, Wv)
    res = run_bass_kernel_spmd(
        nc, in_maps, core_ids=list(range(N_CORES)), trace=trace
    )
    out = np.empty((B, S, D), dtype=np.float32)
    for c in range(N_CORES):
        b, h = divmod(c, 2)
        out[b, h * HALF : (h + 1) * HALF, :] = res.results[c]["out"]
    return out, res


def kernel(x, Wq, Wk, Wv):
    out, _ = _run(
        np.asarray(x, dtype=np.float32),
        np.asarray(Wq, dtype=np.float32),
        np.asarray(Wk, dtype=np.float32),
        np.asarray(Wv, dtype=np.float32),
    )
    return out


# revision 7
# speedup vs baseline: 1.5590x; 1.5590x over previous
"""Single-head attention (B=4, S=4096, E=1024, D=64) on 8 TRN2 NeuronCores.

Sharding: data-parallel over (batch, query-half): core c handles batch
b = c // 2 and query rows [h*2048, (h+1)*2048) with h = c % 2. Each core
computes Q for its own 2048 rows and K/V for the full 4096 rows of its batch
(inputs are shipped host-pretransposed per half, so no duplicated DMA).

Per-core dataflow (TensorE matmuls in bf16 — fp32/fp32r matmuls run the PE
at half clock; fp32 accumulation in PSUM):
  x^T (bf16)  --WqT/WkT/WvT-->  Q^T [64, 2048], K^T [64, S] (bf16),
  V^T_aug [65, S] (fp32, row 64 = ones) -> PE transpose -> V_aug [k, 65] bf16.
  scores^T[k, q] = K^T.T @ Q^T -> exp on ScalarE (scale folded) -> P bf16
  attn^T[65, q] += V_aug.T @ P   (row 64 accumulates softmax denominators)
  final: PE-transpose attn^T tiles, scale rows by reciprocal denominators.

Emission is software-pipelined so ScalarE exp (the second-busiest engine)
starts ~15us in and PE never sits idle waiting for a phase boundary.
"""

import numpy as np

B, S, E, D = 4, 4096, 1024, 64
HALF = S // 2
N_CORES = 8
SCALE = 1.0 / np.sqrt(D)

NE = E // 128  # 8 e-tiles
NKT = S // 128  # 32 k-tiles
NQT = HALF // 128  # 16 q-tiles per core
NCH = S // 512  # 8 proj column-chunks of 512 for K/V

_CACHE = {}


def _build():
    if "nc" in _CACHE:
        return _CACHE["nc"]

    from contextlib import ExitStack

    import concourse.bacc as bacc
    import concourse.tile as tile
    from concourse import mybir
    from concourse.masks import make_identity

    FP32 = mybir.dt.float32
    BF16 = mybir.dt.bfloat16
    Exp = mybir.ActivationFunctionType.Exp

    nc = bacc.Bacc(
        "TRN2", target_bir_lowering=False, debug=False, num_devices=N_CORES
    )

    xt_q_d = nc.dram_tensor("xt_q", [E, HALF], BF16, kind="ExternalInput").ap()
    xt_o_d = nc.dram_tensor("xt_o", [E, HALF], BF16, kind="ExternalInput").ap()
    wt_d = nc.dram_tensor("wt", [E, 3 * D], BF16, kind="ExternalInput").ap()
    out_d = nc.dram_tensor("out", [HALF, D], FP32, kind="ExternalOutput").ap()

    with tile.TileContext(nc) as tc, ExitStack() as ctx:
        const = ctx.enter_context(tc.tile_pool(name="const", bufs=1))
        big = ctx.enter_context(tc.tile_pool(name="big", bufs=1))
        pp = ctx.enter_context(tc.tile_pool(name="pp", bufs=6))
        sm = ctx.enter_context(tc.tile_pool(name="sm", bufs=4))
        psA = ctx.enter_context(tc.tile_pool(name="psA", bufs=2, space="PSUM"))
        psB = ctx.enter_context(tc.tile_pool(name="psB", bufs=1, space="PSUM"))

        ident = const.tile([128, 128], FP32)
        make_identity(nc, ident)

        xt = big.tile([128, NE, S], BF16)  # x^T; cols [0, HALF) = own q-rows
        wt = big.tile([128, NE, 3 * D], BF16)  # WqT | WkT | WvT
        qt = big.tile([64, HALF], BF16)  # Q^T
        kt = big.tile([64, S], BF16)  # K^T
        vt = big.tile([65, S], FP32)  # V^T staging; row 64 = ones
        vn = big.tile([128, NKT, D + 1], BF16)  # V natural + ones column
        att_sb = big.tile([65, HALF], FP32)  # attn^T + denominator row
        out_sb = big.tile([128, NQT, D], FP32)

        # --- input DMAs: 1024-col pieces, own half first (Q path) ---
        for grp in range(2):
            for et in range(NE):
                c0 = grp * 1024
                nc.gpsimd.dma_start(
                    out=xt[:, et, c0 : c0 + 1024],
                    in_=xt_q_d[et * 128 : (et + 1) * 128, c0 : c0 + 1024],
                )
        nc.sync.dma_start(out=wt[:, :, :], in_=wt_d.rearrange("(t p) d -> p t d", p=128))
        for grp in range(2):
            for et in range(NE):
                c0 = grp * 1024
                nc.gpsimd.dma_start(
                    out=xt[:, et, HALF + c0 : HALF + c0 + 1024],
                    in_=xt_o_d[et * 128 : (et + 1) * 128, c0 : c0 + 1024],
                )

        nc.vector.memset(vt[64:65, :], 1.0)

        # one projection column-chunk of 512: dst[:, cols] = (x W_widx)^T
        def proj_chunk(widx, dst, c):
            acc = psA.tile([128, 1024], FP32, tag="ps")
            for et in range(NE):
                nc.tensor.matmul(
                    out=acc[0:64, 0:512],
                    lhsT=wt[:, et, widx * D : (widx + 1) * D],
                    rhs=xt[:, et, c * 512 : (c + 1) * 512],
                    start=(et == 0),
                    stop=(et == NE - 1),
                )
            nc.vector.tensor_copy(
                out=dst[:, c * 512 : (c + 1) * 512], in_=acc[0:64, 0:512]
            )

        def v_transpose(k):
            tp = psA.tile([128, 1024], FP32, tag="ps")
            nc.tensor.transpose(
                out=tp[0:128, 0:65],
                in_=vt[:, k * 128 : (k + 1) * 128],
                identity=ident[0:65, 0:65],
            )
            nc.vector.tensor_copy(out=vn[:, k, :], in_=tp[0:128, 0:65])

        # --- prologue: Q fully; K/V chunks 0-1; V transposes 0-3 ---
        for c in range(HALF // 512):
            proj_chunk(0, qt, c)
        for c in range(2):
            proj_chunk(1, kt, c)
            proj_chunk(2, vt[0:64, :], c)
        for k in range(4):
            v_transpose(k)

        att_ps = psB.tile([128, HALF], FP32)
        p_tiles = {}

        # --- pipelined attention loop ---
        for k in range(NKT):
            if k % 4 == 0 and k // 4 + 2 < NCH:
                proj_chunk(1, kt, k // 4 + 2)
                proj_chunk(2, vt[0:64, :], k // 4 + 2)
            if k + 4 < NKT:
                v_transpose(k + 4)

            for h in range(2):
                sc = psA.tile([128, 1024], FP32, tag="ps")
                for c in range(2):
                    q0 = h * 1024 + c * 512
                    nc.tensor.matmul(
                        out=sc[:, c * 512 : (c + 1) * 512],
                        lhsT=kt[:, k * 128 : (k + 1) * 128],
                        rhs=qt[:, q0 : q0 + 512],
                        start=True,
                        stop=True,
                    )
                p = pp.tile([128, 1024], BF16)
                nc.scalar.activation(out=p[:, :], in_=sc[:, :], func=Exp, scale=SCALE)
                p_tiles[(k, h)] = p

            if k >= 2:
                _attn(nc, att_ps, vn, p_tiles, k - 2)

        _attn(nc, att_ps, vn, p_tiles, NKT - 2)
        _attn(nc, att_ps, vn, p_tiles, NKT - 1)

        # --- normalize + transpose back to [q, d] ---
        nc.vector.tensor_copy(out=att_sb[:, :], in_=att_ps[0:65, :])
        for t in range(NQT):
            tp = psA.tile([128, 1024], FP32, tag="ps")
            nc.tensor.transpose(
                out=tp[0:128, 0:65],
                in_=att_sb[:, t * 128 : (t + 1) * 128],
                identity=ident[0:65, 0:65],
            )
            rc = sm.tile([128, 1], FP32)
            nc.vector.reciprocal(out=rc[:, :], in_=tp[0:128, 64:65])
            nc.vector.tensor_scalar_mul(out_sb[:, t, :], tp[0:128, 0:64], rc[:, :])

        nc.sync.dma_start(
            out=out_d.rearrange("(t p) d -> p t d", p=128), in_=out_sb[:, :, :]
        )

    nc.compile()
    _CACHE["nc"] = nc
    return nc


def _attn(nc, att_ps, vn, p_tiles, k):
    for h in range(2):
        p = p_tiles.pop((k, h))
        for c in range(2):
            q0 = h * 1024 + c * 512
            nc.tensor.matmul(
                out=att_ps[0:65, q0 : q0 + 512],
                lhsT=vn[:, k, :],
                rhs=p[:, c * 512 : (c + 1) * 512],
                start=(k == 0),
                stop=(k == NKT - 1),
                skip_group_check=True,
            )


def _make_in_maps(x, Wq, Wk, Wv):
    import ml_dtypes

    bf16 = ml_dtypes.bfloat16
    xT = np.ascontiguousarray(x.transpose(0, 2, 1)).astype(bf16)  # [B, E, S]
    wt = np.concatenate([Wq.T, Wk.T, Wv.T], axis=1).astype(bf16)  # [E, 3D]
    in_maps = []
    for c in range(N_CORES):
        b, h = divmod(c, 2)
        in_maps.append(
            {
                "xt_q": np.ascontiguousarray(xT[b, :, h * HALF : (h + 1) * HALF]),
                "xt_o": np.ascontiguousarray(
                    xT[b, :, (1 - h) * HALF : (2 - h) * HALF]
                ),
                "wt": wt,
            }
        )
    return in_maps


def _run(x, Wq, Wk, Wv, trace=False):
    from concourse.bass_utils import run_bass_kernel_spmd

    nc = _build()
    in_maps = _make_in_maps(x, Wq, Wk, Wv)
    res = run_bass_kernel_spmd(
        nc, in_maps, core_ids=list(range(N_CORES)), trace=trace
    )
    out = np.empty((B, S, D), dtype=np.float32)
    for c in range(N_CORES):
        b, h = divmod(c, 2)
        out[b, h * HALF : (h + 1) * HALF, :] = res.results[c]["out"]
    return out, res


def kernel(x, Wq, Wk, Wv):
    out, _ = _run(
        np.asarray(x, dtype=np.float32),
        np.asarray(Wq, dtype=np.float32),
        np.asarray(Wk, dtype=np.float32),
        np.asarray(Wv, dtype=np.float32),
    )
    return out
